# revision 68
# baseline (speedup 1.0000x reference)
"""PoPE attention kernel for Trainium2, sharded over 8 NeuronCores by heads.

Problem: B=1, S=2048, DIM=1024, H=16 heads, D=64.
  q/k/v = x @ w{q,k,v}^T ; PoPE embed (softplus magnitude x cos/sin phase);
  scores = q_emb @ k_emb^T / sqrt(D); softmax; out = attn @ v; y = out @ wo^T.

Sharding: 2 heads per core. Each core computes its heads' projections,
attention, and a partial output projection (its 128 channels of wo);
host sums the 8 partial y's (f32) - no on-chip collectives.

Schedule (v2): the kernel is PE-bound (~196k matmul columns ~ 82us at
2.4GHz, ~103us busy with weight loads) with the ACT exp stream (64
chunks of [128,1024] at ~1.0us + ~8.5us softplus) as co-pacer. The
schedule keeps both saturated from the earliest DMA arrival:
  - inputs go out as ~256KB pieces round-robined over the three DGE
    rings (sync/gpsimd/scalar; ~22GB/s per HW queue, ~100GB/s per
    ring). A trigger BLOCKS its sequencer until DGE ring space frees,
    so scalar (ACT) carries only prefix pieces (wqk, btr, tq-lo,
    xt-lo); the tail (xt-hi, wv, tq-hi, wo) rides sync+gpsimd.
  - key-chunk stages are ordered (h0-lo, h1-lo, h0-hi, h1-hi keys) so
    the first 16 exp chunks depend only on the low half of x; the
    high-half projections + softplus slot into the stream while the
    low-key attention runs; h1's qh0 hi-key attn@v spills into qh1's
    PE slack.
  - the Tile scheduler's DMA model is optimistic, so every DMA-gated
    block (qkproj-hi, vproj, tk-hi) carries a tile_wait_until pin
    matching its REAL input-arrival time - otherwise the scheduler
    front-loads it into the static engine order, where it stalls the
    score->exp chain at runtime.
  - the attn@v accumulators, qkproj-hi psums and the v-projection
    time-share the avA/avB PSUM banks between consumers.
  - softmax 1/rowsum runs off-ACT (ones-column rowsum on PE partition
    64, DVE reciprocal via DMA bounce, gpsimd broadcast).
  - tail: last head's attn@v accumulates in two [128,512] PSUM halves,
    each normalized + output-projected as soon as it completes;
    keep-warm junk matmuls hold the PE clock at 2.4GHz through the
    rowsum-chain waits; y DMAs spread over all three rings with a
    6-deep y_sb pool so evictions never wait on drain.
"""
import math

import numpy as np
import ml_dtypes

import concourse.bacc as bacc
import concourse.mybir as mybir
from concourse import tile
from concourse.bass_utils import run_bass_kernel_spmd

BF16 = ml_dtypes.bfloat16
S, DIM, H, D = 2048, 1024, 16, 64
NCORES = 8
HPC = H // NCORES          # heads per core = 2
ED = 2 * D                 # embedding width per head = 128
KI = DIM // 128            # contraction chunks for projections = 8
KC = S // 128              # key-token chunks = 16
OC = DIM // 128            # output-channel chunks = 8
QH = 1024                  # query superblock width

_compiled_nc = None


class _patch_act_tables:
    """Context manager: make bacc's activation-table pass pick
    natural_log_exp_and_others for both Exp and Ln (they otherwise land in
    two different sets and every Exp<->Ln transition costs a ~1.5us
    ACT_TABLE_LOAD). Masking exp/ln out of the smaller sets preserves set
    indices (act_func_set_id is positional) while forcing the combined
    set. Restored on exit so nothing outside this kernel's compile is
    affected."""

    def __enter__(self):
        self._orig = orig = bacc.get_activation_tables

        def patched(arch):
            tabs = dict(orig(arch))
            AF = mybir.ActivationFunctionType
            combined = None
            for name, fns in tabs.items():
                if AF.Exp in fns and AF.Ln in fns:
                    combined = name
                    break
            if combined is None:
                return tabs
            for name, fns in tabs.items():
                if name != combined:
                    tabs[name] = fns - {AF.Exp, AF.Ln}
            return tabs

        bacc.get_activation_tables = patched

    def __exit__(self, *exc):
        bacc.get_activation_tables = self._orig
        return False


def _build_body(nc, tc, persist, ps_pool, out_pool, xt_pool, exp_pool, ext):
    dt = mybir.dt
    AF = mybir.ActivationFunctionType
    ALU = mybir.AluOpType
    xt_ext, wqk_ext, wv_ext, tq_ext, btr_ext, wo_ext, y_ext = ext

    # ---- HAM warmup: dummy matmuls on junk data while the input DMAs run,
    # so the PE clock-gate reaches 2.4 GHz before the real matmuls start ----
    warm_sb = persist.tile([128, 512], dt.bfloat16)
    nc.gpsimd.memset(warm_sb[:], 0.0)
    # v with a ones column appended per (head, key chunk); the softmax
    # rowsum lands on PSUM partition 64 (32-aligned, which the AP hardware
    # requires; ACT ops only work at partition base 0). Memsets run BEFORE
    # gpsimd's DMA triggers so the ring back-pressure can't delay them.
    v_sb = persist.tile([128, HPC, KC, D + 1], dt.bfloat16)
    nc.gpsimd.memset(v_sb[:, 0, :, D], 1.0)
    nc.gpsimd.memset(v_sb[:, 1, :, D], 1.0)

    warm_ps = ps_pool.tile([128, 512], dt.float32, name="warm_ps", tag="scA")
    for i in range(25):
        nc.tensor.matmul(warm_ps[:], warm_sb[:, 0:128], warm_sb[:],
                         start=(i == 0), stop=(i == 24))
    # dummy exp: pulls the 1.3us ACT_TABLE_LOAD to ACT-idle time instead
    # of right before the first softplus exp on the critical path
    warm_act = persist.tile([1, 8], dt.bfloat16, name="warm_act", tag="wact")
    nc.scalar.activation(warm_act[:], warm_sb[0:1, 0:8], AF.Exp)

    # ---- input DMAs ----
    wqk_sb = persist.tile([128, 2, KI, ED], dt.bfloat16)
    tq_sb = persist.tile([128, 2, 2, QH], dt.bfloat16)   # [half, cos/sin]
    btr_sb = persist.tile([128, 2], dt.float32)
    xt = xt_pool.tile([128, 2, KI, QH], dt.bfloat16)     # [half, ki, cols]
    wv_sb = persist.tile([128, KI, ED], dt.bfloat16)
    wo_sb = persist.tile([128, DIM], dt.bfloat16)

    # ~256KB pieces round-robined over the three DGE rings: many pieces
    # in flight keep all 16 HW queues busy (~300GB/s aggregate); a
    # single big piece only reaches ~60-100GB/s. Pieces drain in issue
    # order per ring, so the prefix (everything qkproj-lo/softplus-lo/
    # tk-lo need) goes first and the tail follows automatically.
    prefix = [(wqk_sb[:, 0], wqk_ext[:, 0]), (wqk_sb[:, 1], wqk_ext[:, 1]),
              (btr_sb[:], btr_ext[:]),
              (tq_sb[:, 0, 0], tq_ext[:, 0, 0]),
              (tq_sb[:, 0, 1], tq_ext[:, 0, 1])]
    prefix += [(xt[:, 0, ki], xt_ext[:, 0, ki]) for ki in range(KI)]
    tail = [(xt[:, 1, ki], xt_ext[:, 1, ki]) for ki in range(KI)]
    tail += [(wv_sb[:, 0:4], wv_ext[:, 0:4]), (wv_sb[:, 4:8], wv_ext[:, 4:8])]
    tail += [(tq_sb[:, 1, t], tq_ext[:, 1, t]) for t in range(2)]
    tail += [(wo_sb[:, 0:512], wo_ext[:, 0:512]),
             (wo_sb[:, 512:1024], wo_ext[:, 512:1024])]
    rings = [nc.sync, nc.gpsimd, nc.scalar]
    for i, (dst, src) in enumerate(prefix):
        rings[i % 3].dma_start(dst, src)
    # scalar (ACT) carries NO tail pieces: a trigger blocks its sequencer
    # until ring space frees, and ACT must be free for softplus.
    for i, (dst, src) in enumerate(tail):
        rings[i % 2].dma_start(dst, src)

    # key-phase trig tables built on DVE (idle during the input DMA):
    # ck = cq*cos(b) - sq*sin(b); sk = sq*cos(b) + cq*sin(b). btr is
    # head-major per partition so both heads go in one full-width op.
    tk_sb = persist.tile([128, 2, 2, QH], dt.bfloat16)   # [half, cos/sin]
    tk_tmp = persist.tile([128, QH], dt.bfloat16, name="tktmp", tag="tktmp")
    cb = btr_sb[:, 0:1]
    sb = btr_sb[:, 1:2]

    def tk_build(half):
        cq, sq = tq_sb[:, half, 0], tq_sb[:, half, 1]
        tt = tk_tmp[:]
        nc.vector.tensor_scalar_mul(tt, sq, sb)
        nc.vector.scalar_tensor_tensor(
            tk_sb[:, half, 0], cq, cb, tt, ALU.mult, ALU.subtract)
        nc.vector.tensor_scalar_mul(tt, cq, sb)
        nc.vector.scalar_tensor_tensor(
            tk_sb[:, half, 1], sq, cb, tt, ALU.mult, ALU.add)

    tk_build(0)

    emb_q = [persist.tile([128, S], dt.bfloat16, name=f"embq{h}", tag=f"embq{h}")
             for h in range(HPC)]
    emb_k = [persist.tile([128, S], dt.bfloat16, name=f"embk{h}", tag=f"embk{h}")
             for h in range(HPC)]
    outT = persist.tile([128, S], dt.bfloat16)

    # ---- q/k projections, low half (ki-outer so MMs chase the xt DMA) ----
    psm = {}
    psm[(0, 0)] = ps_pool.tile([128, QH], dt.float32, name="psm00", tag="scA")
    psm[(1, 0)] = ps_pool.tile([128, QH], dt.float32, name="psm10", tag="scB")

    # inner order (k half0, q half0, q half1, k half1): each psum half
    # stops independently at ki=7, and the softplus chain consumes them
    # in exactly this order - the first score's inputs stop first
    for ki in range(KI):
        for p, qc in ((1, 0), (0, 0), (0, 1), (1, 1)):
            nc.tensor.matmul(
                psm[(p, 0)][:, qc * 512:(qc + 1) * 512],
                wqk_sb[:, p, ki, :],
                xt[:, 0, ki, qc * 512:(qc + 1) * 512],
                start=(ki == 0), stop=(ki == KI - 1),
            )

    # softplus(x) = ln(1 + e^x); exp and ln share one activation table
    # (pinned by _patch_act_tables) so no table switches occur.
    tmp = xt_pool.tile([128, 2, S], dt.float32, name="sp", tag="sp")
    mag = xt_pool.tile([128, 2, S], dt.bfloat16, name="mag", tag="mag")
    qk_mag = [mag[:, 0, :], mag[:, 1, :]]

    # softplus in 512-col pieces, k's first half first: the first scores
    # matmul needs only emb_k cols 0-127 + emb_q cols 0-511, so the
    # serial psum->exp->ln->emb->mm chain to the first attention exp
    # shortens by ~1us versus full-width ops.
    for (p, cs) in ((1, slice(0, 512)), (0, slice(0, 512)),
                    (0, slice(512, QH)), (1, slice(512, QH))):
        nc.scalar.activation(tmp[:, p, cs], psm[(p, 0)][:, cs], AF.Exp)
        nc.scalar.activation(mag[:, p, cs], tmp[:, p, cs], AF.Ln, bias=1.0)

    # embeds on DVE (bf16 SBUF 2x mode); head 0's first key chunk split
    # out so the first scores matmul is gated by ~0.5us of DVE work
    def emb_mul(lo, h, qcols=None, kcols=None):
        r = slice(64 * h, 64 * h + 64)
        for t in range(2):  # 0=cos part, 1=sin part
            e = slice(64 * t, 64 * t + 64)
            if kcols is not None:
                gc = slice(lo * QH + kcols.start, lo * QH + kcols.stop)
                nc.vector.tensor_mul(emb_k[h][e, gc], qk_mag[1][r, gc],
                                     tk_sb[r, lo, t, kcols])
            if qcols is not None:
                gq = slice(lo * QH + qcols.start, lo * QH + qcols.stop)
                nc.vector.tensor_mul(emb_q[h][e, gq], qk_mag[0][r, gq],
                                     tq_sb[r, lo, t, qcols])

    full = slice(0, QH)
    emb_mul(0, 0, kcols=slice(0, 128))     # first-score lhsT
    emb_mul(0, 0, qcols=slice(0, 512))     # first-score rhs, half 0
    emb_mul(0, 0, qcols=slice(512, QH))
    emb_mul(0, 0, kcols=slice(128, QH))
    emb_mul(0, 1, qcols=full, kcols=full)

    # ---- v projection (psums borrow avB banks) ----
    def v_group_mm(g, tag):
        psv = ps_pool.tile([128, 4, 128], dt.float32, name=f"psv{g}", tag=tag)
        for sub in range(4):
            t = 4 * g + sub
            half, lc = t // 8, (t % 8) * 128
            for ki in range(KI):
                nc.tensor.matmul(
                    psv[:, sub, :],
                    xt[:, half, ki, lc:lc + 128],
                    wv_sb[:, ki, :],
                    start=(ki == 0), stop=(ki == KI - 1),
                )
        return psv

    def v_group_evict(g, psv):
        for h in range(HPC):
            for sub in range(4):
                t = 4 * g + sub
                nc.vector.tensor_copy(
                    v_sb[:, h, t, 0:D], psv[:, sub, 64 * h:64 * h + 64])

    # ---- attention machinery ----
    av_ps = {}
    exp_tiles = {}

    def scores_chunk(h, kc, qh):
        e = exp_pool.tile([128, QH], dt.bfloat16,
                          name=f"exp{qh}_{h}_{kc}", tag=f"exp{qh}_{kc % 8}")
        exp_tiles[(h, kc, qh)] = e
        sc = ps_pool.tile([128, QH], dt.float32, name=f"sc{qh}_{h}_{kc}",
                          tag=("scA", "scB")[kc % 2])
        for q2 in range(2):
            nc.tensor.matmul(
                sc[:, q2 * 512:(q2 + 1) * 512],
                emb_k[h][:, kc * 128:(kc + 1) * 128],
                emb_q[h][:, qh * QH + q2 * 512:qh * QH + (q2 + 1) * 512],
                start=True, stop=True,
            )
        nc.scalar.activation(e[:], sc[:], AF.Exp, scale=1.0 / math.sqrt(D))

    def av_chunk(h, kc, qh, q2s=(0, 1)):
        e = exp_tiles[(h, kc, qh)]
        for q2 in q2s:
            if h == 1:
                pav, col = av_ps[(1, qh)][q2], slice(0, 512)
            else:
                pav, col = av_ps[(h, qh)], slice(q2 * 512, (q2 + 1) * 512)
            nc.tensor.matmul(
                pav[0:D + 1, col],
                v_sb[:, h, kc, :],
                e[:, q2 * 512:(q2 + 1) * 512],
                start=(kc == 0), stop=(kc == KC - 1),
            )

    def normalize(h, qh):
        # Softmax 1/rowsum off the critical path: rowsum row (PSUM
        # partition 64, thanks to the ones column in v) out first in
        # bf16, DMA-spread over 128 partitions (DVE reciprocal is
        # free-size bound), reciprocal, DMA back, gpsimd broadcast; the
        # attn@v accumulator is evicted bf16 and multiplied in DVE 2x mode.
        if h == 1:
            pavs = [(av_ps[(1, qh)][q2], slice(q2 * 512, (q2 + 1) * 512))
                    for q2 in range(2)]
        else:
            pavs = [(av_ps[(h, qh)], slice(0, QH))]
        rsrow = persist.tile([D + 1, QH], dt.bfloat16, name=f"rsr{h}_{qh}",
                             tag=f"rsr{h}")
        with nc.allow_low_precision(reason="softmax rowsum in bf16 is ~0.4% scale noise"):
            for pav, cs in pavs:
                nc.vector.tensor_copy(rsrow[D:D + 1, cs], pav[D:D + 1, :])
        rs128 = persist.tile([128, QH // 128], dt.bfloat16,
                             name=f"rs128_{h}_{qh}", tag=f"rs128_{h}")
        nc.sync.dma_start(rs128[:], rsrow[D:D + 1, :])
        rr128 = persist.tile([128, QH // 128], dt.bfloat16,
                             name=f"rr128_{h}_{qh}", tag=f"rr128_{h}")
        with nc.allow_low_precision(reason="softmax 1/rowsum in bf16 is ~0.4% scale noise"):
            nc.vector.reciprocal(rr128[:], rs128[:])
        rr = persist.tile([1, QH], dt.bfloat16, name=f"rr{h}_{qh}", tag=f"rr{h}")
        nc.sync.dma_start(rr[:], rr128[:])
        acopy = persist.tile([D, QH], dt.bfloat16,
                             name=f"acopy{h}_{qh}", tag=f"acopy{h}")
        with nc.allow_low_precision(reason="pre-normalize attn@v in bf16, ~0.4%"):
            for pav, cs in pavs:
                nc.vector.tensor_copy(acopy[:, cs], pav[0:D, :])
        rsb = persist.tile([D, QH], dt.bfloat16, name=f"rsb{h}_{qh}",
                           tag=f"rsb{h}")
        nc.gpsimd.partition_broadcast(rsb[:], rr[:])
        nc.vector.tensor_mul(outT[64 * h:64 * h + 64, qh * QH:(qh + 1) * QH],
                             acopy[:], rsb[:])

    def oproj(qh, oc):
        # output projection for superblock qh, channel chunk oc, in two
        # [128,512] PSUM halves (avB0/avB1); evictions stay on DVE.
        c = slice(qh * QH, (qh + 1) * QH)
        y_sb = out_pool.tile([128, QH], dt.bfloat16, name=f"y{qh}_{oc}", tag="y")
        for q2 in range(2):
            psy = ps_pool.tile([128, 512], dt.float32, name=f"psy{qh}_{oc}{q2}",
                               tag=f"avB{q2}")
            nc.tensor.matmul(
                psy[:],
                wo_sb[:, oc * 128:(oc + 1) * 128],
                outT[:, qh * QH + q2 * 512:qh * QH + (q2 + 1) * 512],
                start=True, stop=True,
            )
            nc.vector.tensor_copy(y_sb[:, q2 * 512:(q2 + 1) * 512], psy[:])
        (nc.sync, nc.gpsimd)[oc % 2].dma_start(y_ext[oc, :, c], y_sb[:])

    # ======== qh=0 superblock (queries 0-1023) ========
    # s0: scores/exp h0 x low keys. The qkproj-hi matmuls and vproj-lo
    # fill the PE slack inside the stream. All DMA-gated blocks carry
    # tile_wait_until pins matching their REAL input-arrival times: the
    # tile scheduler's DMA model is optimistic, and without a pin it
    # front-loads these into the static engine order, where they then
    # stall the score->exp chain at runtime.
    for j in range(8):
        scores_chunk(0, j, 0)

    # qkproj-hi: k -> avA (freed first by softplus-hi's k exp, unblocking
    # the av(h0) accumulator), q -> avB halves. Pinned late enough that
    # the runtime xt-hi arrival (~26us) can't stall the score stream.
    psm[(1, 1)] = ps_pool.tile([128, QH], dt.float32, name="psm11", tag="avA")
    psm[(0, 1)] = [
        ps_pool.tile([128, 512], dt.float32, name=f"psm01_{q2}", tag=f"avB{q2}")
        for q2 in range(2)]
    for ki in range(KI):
        with tc.tile_wait_until(0.0145 + 0.002 * (ki // 2)):
            for p in (1, 0):
                for qc in range(2):
                    t = psm[(p, 1)]
                    if isinstance(t, list):
                        dst = t[qc][:, 0:512]
                    else:
                        dst = t[:, qc * 512:qc * 512 + 512]
                    nc.tensor.matmul(
                        dst,
                        wqk_sb[:, p, ki, :],
                        xt[:, 1, ki, qc * 512:(qc + 1) * 512],
                        start=(ki == 0), stop=(ki == KI - 1),
                    )

    # ACT: softplus-hi slots into the exp stream (k first: avA frees for
    # the av(h0) accumulator immediately).
    nc.scalar.activation(tmp[:, 1, QH:S], psm[(1, 1)][:], AF.Exp)
    for q2 in range(2):
        nc.scalar.activation(tmp[:, 0, QH + q2 * 512:QH + (q2 + 1) * 512],
                             psm[(0, 1)][q2][:], AF.Exp)
    nc.scalar.activation(mag[:, :, QH:S], tmp[:, :, QH:S], AF.Ln, bias=1.0)

    # DVE: hi-half trig + embeds (h0 first - its hi-key scores come at s2)
    with tc.tile_wait_until(0.028):
        tk_build(1)
    emb_mul(1, 0, qcols=full, kcols=full)
    emb_mul(1, 1, qcols=full, kcols=full)

    # s1: scores h1 x low keys only - the attn@v chunks are redistributed
    # into s2..qh1-s1, because v (whose wv input only lands ~27us) isn't
    # evicted until ~32us.
    for j in range(8):
        scores_chunk(1, j, 0)

    # vproj: wv lands ~27us; the avB banks free up once softplus-hi's q
    # exp has read the qkproj-hi psums.
    with tc.tile_wait_until(0.026):
        psv0 = v_group_mm(0, "avB0")
        psv1 = v_group_mm(1, "avB1")
    with tc.tile_wait_until(0.029):
        v_group_evict(0, psv0)
        v_group_evict(1, psv1)
    with tc.tile_wait_until(0.032):
        psv2 = v_group_mm(2, "avB0")
        psv3 = v_group_mm(3, "avB1")
    with tc.tile_wait_until(0.035):
        v_group_evict(2, psv2)
        v_group_evict(3, psv3)

    # s2: av(h0) x low keys + scores h0 x high keys
    av_ps[(0, 0)] = ps_pool.tile([128, QH], dt.float32, name="av0_0", tag="avA")
    for j in range(8):
        av_chunk(0, j, 0)
        scores_chunk(0, 8 + j, 0)

    # s3: av(h1) x low keys + scores h1 x high keys + av(h0) hi (lag 1)
    av_ps[(1, 0)] = [
        ps_pool.tile([128, 512], dt.float32, name=f"av1_0{q2}", tag=f"avB{q2}")
        for q2 in range(2)]
    for j in range(8):
        av_chunk(1, j, 0)
        scores_chunk(1, 8 + j, 0)
        if j >= 1:
            av_chunk(0, 8 + j - 1, 0)
    av_chunk(0, 15, 0)
    normalize(0, 0)

    # ======== qh=1 superblock (queries 1024-2047) ========
    # s0: scores h0 x low keys + av(h1) x qh0 high keys (lag 1); h1's
    # qh0 attn@v spills into qh1's PE slack instead of overcommitting s3
    for j in range(8):
        scores_chunk(0, j, 1)
        if j >= 1:
            av_chunk(1, 8 + j - 1, 0)
    av_chunk(1, 15, 0)
    normalize(1, 0)
    # s1: scores h1-lo + av(h0)-lo + the qh=0 output projection
    av_ps[(0, 1)] = ps_pool.tile([128, QH], dt.float32, name="av0_1", tag="avA")
    for j in range(8):
        scores_chunk(1, j, 1)
        av_chunk(0, j, 1)
        oproj(0, j)
    # s2: scores h0-hi + av(h1)-lo (both column halves)
    av_ps[(1, 1)] = [
        ps_pool.tile([128, 512], dt.float32, name=f"av1_1{q2}", tag=f"avB{q2}")
        for q2 in range(2)]
    for j in range(8):
        scores_chunk(0, 8 + j, 1)
        av_chunk(1, j, 1)
    # s3: scores h1-hi + av(h0)-hi + av(h1)-hi half 0 (lag 1), so half
    # 0's accumulator completes right after the last exp and its
    # normalize chain starts immediately; half 1's hi keys are deferred
    # to the tail as PE filler for the rowsum chain.
    for j in range(8):
        scores_chunk(1, 8 + j, 1)
        av_chunk(0, 8 + j, 1)
        if j >= 1:
            av_chunk(1, 8 + j - 1, 1, q2s=(0,))
    normalize(0, 1)
    av_chunk(1, 15, 1, q2s=(0,))

    # ---- tail. Both column halves' 1/rowsum chains pipeline across
    # DVE/DMA/ACT/GpSimd; the final output projection chases per half in
    # [128,512] pieces with evictions split DVE/ACT. ----
    rrf = persist.tile([1, QH], dt.bfloat16, name="rrf", tag="rrf")
    rsbf = persist.tile([D, QH], dt.bfloat16, name="rsbf", tag="rsbf")

    def norm_final_evict(hf):
        pavh = av_ps[(1, 1)][hf]
        cl = slice(hf * 512, (hf + 1) * 512)
        lnt = persist.tile([1, 512], dt.float32, name=f"lnt{hf}", tag=f"lnt{hf}")
        # ACT reads the PSUM rowsum row (partition 64) directly, writing to
        # partition 0 - skips the SBUF bounce + DMA relocation hop
        nc.scalar.activation(lnt[:], pavh[D:D + 1, :], AF.Ln)
        with nc.allow_low_precision(reason="softmax 1/rowsum in bf16, ~0.4%"):
            nc.scalar.activation(rrf[:, cl], lnt[:], AF.Exp, scale=-1.0)
        nc.gpsimd.partition_broadcast(rsbf[:, cl], rrf[:, cl])

    def norm_final_mul(hf):
        pavh = av_ps[(1, 1)][hf]
        cl = slice(hf * 512, (hf + 1) * 512)
        with nc.allow_low_precision(reason="pre-normalize attn@v in bf16, ~0.4%"):
            nc.vector.tensor_mul(
                outT[64:128, QH + hf * 512:QH + (hf + 1) * 512],
                pavh[0:D, :], rsbf[:, cl])

    def oproj_final_half(hf, tags):
        for oc in range(OC):
            psy = ps_pool.tile([128, 512], dt.float32, name=f"psyF{hf}_{oc}",
                               tag=tags[oc % len(tags)])
            nc.tensor.matmul(
                psy[:],
                wo_sb[:, oc * 128:(oc + 1) * 128],
                outT[:, QH + hf * 512:QH + (hf + 1) * 512],
                start=True, stop=True,
            )
            y_sb = out_pool.tile([128, 512], dt.bfloat16,
                                 name=f"yF{hf}_{oc}", tag="y")
            if oc % 2 == 0:
                nc.vector.tensor_copy(y_sb[:], psy[:])
            else:
                nc.scalar.activation(y_sb[:], psy[:], AF.Copy)
            (nc.sync, nc.gpsimd, nc.scalar)[oc % 3].dma_start(
                y_ext[oc, :, QH + hf * 512:QH + (hf + 1) * 512], y_sb[:])

    # PE fills the half-0 rowsum-chain window with half-1's attn@v; each
    # half's normalize + output projection starts as soon as ITS
    # accumulator (a separate PSUM tile) completes. Junk matmuls plug
    # the rowsum-chain waits so the PE clock-gate never drops off 2.4
    # GHz (an idle gap costs 2x on every tail matmul after it).
    def keep_warm(n, tag="scA"):
        wps = ps_pool.tile([128, 512], dt.float32, name=f"warmF{keep_warm.i}",
                           tag=tag)
        keep_warm.i += 1
        for i in range(n):
            nc.tensor.matmul(wps[:], warm_sb[:, 0:128], warm_sb[:],
                             start=(i == 0), stop=(i == n - 1))
    keep_warm.i = 0

    norm_final_evict(0)
    for kc in range(8, KC):
        av_chunk(1, kc, 1, q2s=(1,))
        keep_warm(1)
    norm_final_evict(1)
    # fill the rowsum-chain window (last av -> outT ready, ~3us) so the
    # PE clock-gate holds 2.4GHz into the final output projection
    keep_warm(12)
    norm_final_mul(0)
    oproj_final_half(0, ("scB", "avA", "scA"))
    norm_final_mul(1)
    oproj_final_half(1, ("scB", "avA", "scA", "avB0"))


def _build():
    with _patch_act_tables():
        return _build_inner()


def _build_inner():
    nc = bacc.Bacc()
    dt = mybir.dt

    ext = (
        nc.declare_dram_parameter("xt", [128, 2, KI, QH], dt.bfloat16, isOutput=False),
        nc.declare_dram_parameter("wqk", [128, 2, KI, ED], dt.bfloat16, isOutput=False),
        nc.declare_dram_parameter("wv", [128, KI, ED], dt.bfloat16, isOutput=False),
        nc.declare_dram_parameter("tq", [128, 2, 2, QH], dt.bfloat16, isOutput=False),
        nc.declare_dram_parameter("btr", [128, 2], dt.float32, isOutput=False),
        nc.declare_dram_parameter("woT", [128, DIM], dt.bfloat16, isOutput=False),
        nc.declare_dram_parameter("yT", [OC, 128, S], dt.bfloat16, isOutput=True),
    )

    with tile.TileContext(nc) as tc:
        with tc.tile_pool(name="persist", bufs=1) as persist, \
             tc.tile_pool(name="ps", bufs=1, space="PSUM") as ps_pool, \
             tc.tile_pool(name="out", bufs=6) as out_pool, \
             tc.tile_pool(name="xtp", bufs=1) as xt_pool, \
             tc.tile_pool(name="expp", bufs=2) as exp_pool:
            _build_body(nc, tc, persist, ps_pool, out_pool, xt_pool, exp_pool, ext)
    nc.compile()
    return nc


def _get_nc():
    global _compiled_nc
    if _compiled_nc is None:
        _compiled_nc = _build()
    return _compiled_nc


def _prep_inputs(x, wq, wk, wv, wo, pope_bias):
    """Host-side sharding + layout prep. Returns in_maps for the 8 cores."""
    x2 = np.ascontiguousarray(x.reshape(S, DIM).astype(np.float32))

    # trig tables (f64 phases for accuracy); heads share cos/sin(freqs)
    inv = 10000.0 ** (-(np.arange(D, dtype=np.float64) / D))
    pos = np.arange(S, dtype=np.float64)
    freqs = pos[:, None] * inv[None, :]                       # [S, D]
    bias = np.clip(pope_bias.astype(np.float64), -2 * math.pi, 0.0)  # [H, D]

    # tq[p, half, t, c]: cos/sin(freqs[half*QH + c, p%64])
    tq = np.empty((128, 2, 2, QH), BF16)
    ct = np.cos(freqs).T.reshape(D, 2, QH)                    # [64, half, QH]
    st = np.sin(freqs).T.reshape(D, 2, QH)
    tq[0:64, :, 0] = ct
    tq[64:128, :, 0] = ct
    tq[0:64, :, 1] = st
    tq[64:128, :, 1] = st

    # xt[q, half, ki, c] = x[half*QH + c, ki*128 + q]; 16KB contiguous
    # per (q, half) so the DMA moves whole halves as single big-run pieces
    xt = np.ascontiguousarray(
        x2.T.reshape(KI, 128, 2, QH).transpose(1, 2, 0, 3)).astype(BF16)

    in_maps = []
    for c in range(NCORES):
        hs = slice(c * HPC * D, (c + 1) * HPC * D)            # head-channel slice
        wqk = np.empty((128, 2, KI, ED), BF16)
        for p, wm in enumerate((wq, wk)):
            wt = np.ascontiguousarray(wm[hs, :].astype(np.float32).T)  # [DIM, ED]
            wqk[:, p] = wt.reshape(KI, 128, ED).transpose(1, 0, 2)
        wvt = np.ascontiguousarray(wv[hs, :].astype(np.float32).T)
        wv_m = np.ascontiguousarray(
            wvt.reshape(KI, 128, ED).transpose(1, 0, 2)).astype(BF16)

        # per-head bias trig for on-chip angle addition, head-major rows:
        # partition 64h+d holds (cos(bias[h,d]), sin(bias[h,d]))
        btr = np.empty((128, 2), np.float32)
        for h in range(HPC):
            b = bias[c * HPC + h]                              # [D]
            btr[64 * h:64 * h + 64, 0] = np.cos(b)
            btr[64 * h:64 * h + 64, 1] = np.sin(b)

        woT = np.ascontiguousarray(wo[:, hs].astype(np.float32).T).astype(BF16)

        in_maps.append({
            "xt": xt, "wqk": wqk, "wv": wv_m, "tq": tq, "btr": btr,
            "woT": woT,
        })
    return in_maps


def kernel(x, wq, wk, wv, wo, pope_bias):
    nc = _get_nc()
    in_maps = _prep_inputs(np.asarray(x), np.asarray(wq), np.asarray(wk),
                           np.asarray(wv), np.asarray(wo), np.asarray(pope_bias))
    res = run_bass_kernel_spmd(nc, in_maps, list(range(NCORES)))
    y = np.zeros((DIM, S), np.float32)
    for c in range(NCORES):
        y += res.results[c]["yT"].reshape(DIM, S).astype(np.float32)
    return np.ascontiguousarray(y.T).reshape(1, S, DIM)


if __name__ == "__main__":
    rng = np.random.default_rng(0)
    out = kernel(
        x=rng.standard_normal((1, S, DIM)).astype(np.float32),
        wq=rng.standard_normal((DIM, DIM)).astype(np.float32) / 32,
        wk=rng.standard_normal((DIM, DIM)).astype(np.float32) / 32,
        wv=rng.standard_normal((DIM, DIM)).astype(np.float32) / 32,
        wo=rng.standard_normal((DIM, DIM)).astype(np.float32) / 32,
        pope_bias=-rng.random((H, D), np.float32) * 3.0,
    )
    print("out", out.shape, out.dtype, np.abs(out).mean())

# revision 69
# speedup vs baseline: 1.0156x; 1.0156x over previous
"""PoPE attention kernel for Trainium2, sharded over 8 NeuronCores by heads.

Problem: B=1, S=2048, DIM=1024, H=16 heads, D=64.
  q/k/v = x @ w{q,k,v}^T ; PoPE embed (softplus magnitude x cos/sin phase);
  scores = q_emb @ k_emb^T / sqrt(D); softmax; out = attn @ v; y = out @ wo^T.

Sharding: 2 heads per core. Each core computes its heads' projections,
attention, and a partial output projection (its 128 channels of wo);
host sums the 8 partial y's (f32) - no on-chip collectives.

Schedule (v2): the kernel is PE-bound (~196k matmul columns ~ 82us at
2.4GHz, ~103us busy with weight loads) with the ACT exp stream (64
chunks of [128,1024] at ~1.0us + ~8.5us softplus) as co-pacer. The
schedule keeps both saturated from the earliest DMA arrival:
  - inputs go out as ~256KB pieces round-robined over the three DGE
    rings (sync/gpsimd/scalar; ~22GB/s per HW queue, ~100GB/s per
    ring). A trigger BLOCKS its sequencer until DGE ring space frees,
    so scalar (ACT) carries only prefix pieces (wqk, btr, tq-lo,
    xt-lo); the tail (xt-hi, wv, tq-hi, wo) rides sync+gpsimd.
  - key-chunk stages are ordered (h0-lo, h1-lo, h0-hi, h1-hi keys) so
    the first 16 exp chunks depend only on the low half of x; the
    high-half projections + softplus slot into the stream while the
    low-key attention runs; h1's qh0 hi-key attn@v spills into qh1's
    PE slack.
  - the Tile scheduler's DMA model is optimistic, so every DMA-gated
    block (qkproj-hi, vproj, tk-hi) carries a tile_wait_until pin
    matching its REAL input-arrival time - otherwise the scheduler
    front-loads it into the static engine order, where it stalls the
    score->exp chain at runtime.
  - the attn@v accumulators, qkproj-hi psums and the v-projection
    time-share the avA/avB PSUM banks between consumers.
  - softmax 1/rowsum runs off-ACT (ones-column rowsum on PE partition
    64, DVE reciprocal via DMA bounce, gpsimd broadcast).
  - tail: last head's attn@v accumulates in two [128,512] PSUM halves,
    each normalized + output-projected as soon as it completes;
    keep-warm junk matmuls hold the PE clock at 2.4GHz through the
    rowsum-chain waits; y DMAs spread over all three rings with a
    6-deep y_sb pool so evictions never wait on drain.
"""
import math

import numpy as np
import ml_dtypes

import concourse.bacc as bacc
import concourse.mybir as mybir
from concourse import tile
from concourse.bass_utils import run_bass_kernel_spmd

BF16 = ml_dtypes.bfloat16
S, DIM, H, D = 2048, 1024, 16, 64
NCORES = 8
HPC = H // NCORES          # heads per core = 2
ED = 2 * D                 # embedding width per head = 128
KI = DIM // 128            # contraction chunks for projections = 8
KC = S // 128              # key-token chunks = 16
OC = DIM // 128            # output-channel chunks = 8
QH = 1024                  # query superblock width

_compiled_nc = None


class _patch_act_tables:
    """Context manager: make bacc's activation-table pass pick
    natural_log_exp_and_others for both Exp and Ln (they otherwise land in
    two different sets and every Exp<->Ln transition costs a ~1.5us
    ACT_TABLE_LOAD). Masking exp/ln out of the smaller sets preserves set
    indices (act_func_set_id is positional) while forcing the combined
    set. Restored on exit so nothing outside this kernel's compile is
    affected."""

    def __enter__(self):
        self._orig = orig = bacc.get_activation_tables

        def patched(arch):
            tabs = dict(orig(arch))
            AF = mybir.ActivationFunctionType
            combined = None
            for name, fns in tabs.items():
                if AF.Exp in fns and AF.Ln in fns:
                    combined = name
                    break
            if combined is None:
                return tabs
            for name, fns in tabs.items():
                if name != combined:
                    tabs[name] = fns - {AF.Exp, AF.Ln}
            return tabs

        bacc.get_activation_tables = patched

    def __exit__(self, *exc):
        bacc.get_activation_tables = self._orig
        return False


def _build_body(nc, tc, persist, ps_pool, out_pool, xt_pool, exp_pool, ext):
    dt = mybir.dt
    AF = mybir.ActivationFunctionType
    ALU = mybir.AluOpType
    xt_ext, wqk_ext, wv_ext, tq_ext, btr_ext, wo_ext, y_ext = ext

    # ---- HAM warmup: dummy matmuls on junk data while the input DMAs run,
    # so the PE clock-gate reaches 2.4 GHz before the real matmuls start ----
    warm_sb = persist.tile([128, 512], dt.bfloat16)
    nc.gpsimd.memset(warm_sb[:], 0.0)
    # v with a ones column appended per (head, key chunk); the softmax
    # rowsum lands on PSUM partition 64 (32-aligned, which the AP hardware
    # requires; ACT ops only work at partition base 0). Memsets run BEFORE
    # gpsimd's DMA triggers so the ring back-pressure can't delay them.
    v_sb = persist.tile([128, HPC, KC, D + 1], dt.bfloat16)
    nc.gpsimd.memset(v_sb[:, 0, :, D], 1.0)
    nc.gpsimd.memset(v_sb[:, 1, :, D], 1.0)

    warm_ps = ps_pool.tile([128, 512], dt.float32, name="warm_ps", tag="scA")
    for i in range(25):
        nc.tensor.matmul(warm_ps[:], warm_sb[:, 0:128], warm_sb[:],
                         start=(i == 0), stop=(i == 24))
    # dummy exp: pulls the 1.3us ACT_TABLE_LOAD to ACT-idle time instead
    # of right before the first softplus exp on the critical path
    warm_act = persist.tile([1, 8], dt.bfloat16, name="warm_act", tag="wact")
    nc.scalar.activation(warm_act[:], warm_sb[0:1, 0:8], AF.Exp)

    # ---- input DMAs ----
    wqk_sb = persist.tile([128, 2, KI, ED], dt.bfloat16)
    tq_sb = persist.tile([128, 2, 2, QH], dt.bfloat16)   # [half, cos/sin]
    btr_sb = persist.tile([128, 2], dt.float32)
    xt = xt_pool.tile([128, 2, KI, QH], dt.bfloat16)     # [half, ki, cols]
    wv_sb = persist.tile([128, KI, ED], dt.bfloat16)
    wo_sb = persist.tile([128, DIM], dt.bfloat16)

    # ~256KB pieces round-robined over the three DGE rings: many pieces
    # in flight keep all 16 HW queues busy (~300GB/s aggregate); a
    # single big piece only reaches ~60-100GB/s. Pieces drain in issue
    # order per ring, so the prefix (everything qkproj-lo/softplus-lo/
    # tk-lo need) goes first and the tail follows automatically.
    prefix = [(wqk_sb[:, 0], wqk_ext[:, 0]), (wqk_sb[:, 1], wqk_ext[:, 1]),
              (btr_sb[:], btr_ext[:]),
              (tq_sb[:, 0, 0], tq_ext[:, 0, 0]),
              (tq_sb[:, 0, 1], tq_ext[:, 0, 1])]
    prefix += [(xt[:, 0, ki], xt_ext[:, 0, ki]) for ki in range(KI)]
    tail = [(xt[:, 1, ki], xt_ext[:, 1, ki]) for ki in range(KI)]
    tail += [(wv_sb[:, 0:4], wv_ext[:, 0:4]), (wv_sb[:, 4:8], wv_ext[:, 4:8])]
    tail += [(tq_sb[:, 1, t], tq_ext[:, 1, t]) for t in range(2)]
    tail += [(wo_sb[:, 0:512], wo_ext[:, 0:512]),
             (wo_sb[:, 512:1024], wo_ext[:, 512:1024])]
    rings = [nc.sync, nc.gpsimd, nc.scalar]
    for i, (dst, src) in enumerate(prefix):
        rings[i % 3].dma_start(dst, src)
    # scalar (ACT) carries NO tail pieces: a trigger blocks its sequencer
    # until ring space frees, and ACT must be free for softplus.
    for i, (dst, src) in enumerate(tail):
        rings[i % 2].dma_start(dst, src)

    # key-phase trig tables built on DVE (idle during the input DMA):
    # ck = cq*cos(b) - sq*sin(b); sk = sq*cos(b) + cq*sin(b). btr is
    # head-major per partition so both heads go in one full-width op.
    tk_sb = persist.tile([128, 2, 2, QH], dt.bfloat16)   # [half, cos/sin]
    tk_tmp = persist.tile([128, QH], dt.bfloat16, name="tktmp", tag="tktmp")
    cb = btr_sb[:, 0:1]
    sb = btr_sb[:, 1:2]

    def tk_build(half):
        cq, sq = tq_sb[:, half, 0], tq_sb[:, half, 1]
        tt = tk_tmp[:]
        nc.vector.tensor_scalar_mul(tt, sq, sb)
        nc.vector.scalar_tensor_tensor(
            tk_sb[:, half, 0], cq, cb, tt, ALU.mult, ALU.subtract)
        nc.vector.tensor_scalar_mul(tt, cq, sb)
        nc.vector.scalar_tensor_tensor(
            tk_sb[:, half, 1], sq, cb, tt, ALU.mult, ALU.add)

    tk_build(0)

    emb_q = [persist.tile([128, S], dt.bfloat16, name=f"embq{h}", tag=f"embq{h}")
             for h in range(HPC)]
    emb_k = [persist.tile([128, S], dt.bfloat16, name=f"embk{h}", tag=f"embk{h}")
             for h in range(HPC)]
    outT = persist.tile([128, S], dt.bfloat16)

    # ---- q/k projections, low half (ki-outer so MMs chase the xt DMA) ----
    psm = {}
    psm[(0, 0)] = ps_pool.tile([128, QH], dt.float32, name="psm00", tag="scA")
    psm[(1, 0)] = ps_pool.tile([128, QH], dt.float32, name="psm10", tag="scB")

    for ki in range(KI):
        for p in range(2):
            for qc in range(2):
                nc.tensor.matmul(
                    psm[(p, 0)][:, qc * 512:(qc + 1) * 512],
                    wqk_sb[:, p, ki, :],
                    xt[:, 0, ki, qc * 512:(qc + 1) * 512],
                    start=(ki == 0), stop=(ki == KI - 1),
                )

    # softplus(x) = ln(1 + e^x); exp and ln share one activation table
    # (pinned by _patch_act_tables) so no table switches occur.
    tmp = xt_pool.tile([128, 2, S], dt.float32, name="sp", tag="sp")
    mag = xt_pool.tile([128, 2, S], dt.bfloat16, name="mag", tag="mag")
    qk_mag = [mag[:, 0, :], mag[:, 1, :]]

    # softplus in 512-col pieces, k's first half first: the first scores
    # matmul needs only emb_k cols 0-127 + emb_q cols 0-511, so the
    # serial psum->exp->ln->emb->mm chain to the first attention exp
    # shortens by ~1us versus full-width ops.
    for (p, cs) in ((1, slice(0, 512)), (0, slice(0, 512)),
                    (0, slice(512, QH)), (1, slice(512, QH))):
        nc.scalar.activation(tmp[:, p, cs], psm[(p, 0)][:, cs], AF.Exp)
        nc.scalar.activation(mag[:, p, cs], tmp[:, p, cs], AF.Ln, bias=1.0)

    # embeds on DVE (bf16 SBUF 2x mode); head 0's first key chunk split
    # out so the first scores matmul is gated by ~0.5us of DVE work
    def emb_mul(lo, h, qcols=None, kcols=None):
        r = slice(64 * h, 64 * h + 64)
        for t in range(2):  # 0=cos part, 1=sin part
            e = slice(64 * t, 64 * t + 64)
            if kcols is not None:
                gc = slice(lo * QH + kcols.start, lo * QH + kcols.stop)
                nc.vector.tensor_mul(emb_k[h][e, gc], qk_mag[1][r, gc],
                                     tk_sb[r, lo, t, kcols])
            if qcols is not None:
                gq = slice(lo * QH + qcols.start, lo * QH + qcols.stop)
                nc.vector.tensor_mul(emb_q[h][e, gq], qk_mag[0][r, gq],
                                     tq_sb[r, lo, t, qcols])

    full = slice(0, QH)
    emb_mul(0, 0, kcols=slice(0, 128))     # first-score lhsT
    emb_mul(0, 0, qcols=slice(0, 512))     # first-score rhs, half 0
    emb_mul(0, 0, qcols=slice(512, QH))
    emb_mul(0, 0, kcols=slice(128, QH))
    emb_mul(0, 1, qcols=full, kcols=full)

    # ---- v projection (psums borrow avB banks) ----
    def v_group_mm(g, tag):
        psv = ps_pool.tile([128, 4, 128], dt.float32, name=f"psv{g}", tag=tag)
        for sub in range(4):
            t = 4 * g + sub
            half, lc = t // 8, (t % 8) * 128
            for ki in range(KI):
                nc.tensor.matmul(
                    psv[:, sub, :],
                    xt[:, half, ki, lc:lc + 128],
                    wv_sb[:, ki, :],
                    start=(ki == 0), stop=(ki == KI - 1),
                )
        return psv

    def v_group_evict(g, psv):
        for h in range(HPC):
            for sub in range(4):
                t = 4 * g + sub
                nc.vector.tensor_copy(
                    v_sb[:, h, t, 0:D], psv[:, sub, 64 * h:64 * h + 64])

    # ---- attention machinery ----
    av_ps = {}
    exp_tiles = {}

    def scores_chunk(h, kc, qh):
        e = exp_pool.tile([128, QH], dt.bfloat16,
                          name=f"exp{qh}_{h}_{kc}", tag=f"exp{qh}_{kc % 8}")
        exp_tiles[(h, kc, qh)] = e
        sc = ps_pool.tile([128, QH], dt.float32, name=f"sc{qh}_{h}_{kc}",
                          tag=("scA", "scB")[kc % 2])
        for q2 in range(2):
            nc.tensor.matmul(
                sc[:, q2 * 512:(q2 + 1) * 512],
                emb_k[h][:, kc * 128:(kc + 1) * 128],
                emb_q[h][:, qh * QH + q2 * 512:qh * QH + (q2 + 1) * 512],
                start=True, stop=True,
            )
        nc.scalar.activation(e[:], sc[:], AF.Exp, scale=1.0 / math.sqrt(D))

    def av_chunk(h, kc, qh, q2s=(0, 1)):
        e = exp_tiles[(h, kc, qh)]
        for q2 in q2s:
            if h == 1:
                pav, col = av_ps[(1, qh)][q2], slice(0, 512)
            else:
                pav, col = av_ps[(h, qh)], slice(q2 * 512, (q2 + 1) * 512)
            nc.tensor.matmul(
                pav[0:D + 1, col],
                v_sb[:, h, kc, :],
                e[:, q2 * 512:(q2 + 1) * 512],
                start=(kc == 0), stop=(kc == KC - 1),
            )

    def normalize(h, qh):
        # Softmax 1/rowsum off the critical path: rowsum row (PSUM
        # partition 64, thanks to the ones column in v) out first in
        # bf16, DMA-spread over 128 partitions (DVE reciprocal is
        # free-size bound), reciprocal, DMA back, gpsimd broadcast; the
        # attn@v accumulator is evicted bf16 and multiplied in DVE 2x mode.
        if h == 1:
            pavs = [(av_ps[(1, qh)][q2], slice(q2 * 512, (q2 + 1) * 512))
                    for q2 in range(2)]
        else:
            pavs = [(av_ps[(h, qh)], slice(0, QH))]
        rsrow = persist.tile([D + 1, QH], dt.bfloat16, name=f"rsr{h}_{qh}",
                             tag=f"rsr{h}")
        with nc.allow_low_precision(reason="softmax rowsum in bf16 is ~0.4% scale noise"):
            for pav, cs in pavs:
                nc.vector.tensor_copy(rsrow[D:D + 1, cs], pav[D:D + 1, :])
        rs128 = persist.tile([128, QH // 128], dt.bfloat16,
                             name=f"rs128_{h}_{qh}", tag=f"rs128_{h}")
        nc.sync.dma_start(rs128[:], rsrow[D:D + 1, :])
        rr128 = persist.tile([128, QH // 128], dt.bfloat16,
                             name=f"rr128_{h}_{qh}", tag=f"rr128_{h}")
        with nc.allow_low_precision(reason="softmax 1/rowsum in bf16 is ~0.4% scale noise"):
            nc.vector.reciprocal(rr128[:], rs128[:])
        rr = persist.tile([1, QH], dt.bfloat16, name=f"rr{h}_{qh}", tag=f"rr{h}")
        nc.sync.dma_start(rr[:], rr128[:])
        acopy = persist.tile([D, QH], dt.bfloat16,
                             name=f"acopy{h}_{qh}", tag=f"acopy{h}")
        with nc.allow_low_precision(reason="pre-normalize attn@v in bf16, ~0.4%"):
            for pav, cs in pavs:
                nc.vector.tensor_copy(acopy[:, cs], pav[0:D, :])
        rsb = persist.tile([D, QH], dt.bfloat16, name=f"rsb{h}_{qh}",
                           tag=f"rsb{h}")
        nc.gpsimd.partition_broadcast(rsb[:], rr[:])
        nc.vector.tensor_mul(outT[64 * h:64 * h + 64, qh * QH:(qh + 1) * QH],
                             acopy[:], rsb[:])

    def oproj(qh, oc):
        # output projection for superblock qh, channel chunk oc, in two
        # [128,512] PSUM halves (avB0/avB1); evictions stay on DVE.
        c = slice(qh * QH, (qh + 1) * QH)
        y_sb = out_pool.tile([128, QH], dt.bfloat16, name=f"y{qh}_{oc}", tag="y")
        for q2 in range(2):
            psy = ps_pool.tile([128, 512], dt.float32, name=f"psy{qh}_{oc}{q2}",
                               tag=f"avB{q2}")
            nc.tensor.matmul(
                psy[:],
                wo_sb[:, oc * 128:(oc + 1) * 128],
                outT[:, qh * QH + q2 * 512:qh * QH + (q2 + 1) * 512],
                start=True, stop=True,
            )
            nc.vector.tensor_copy(y_sb[:, q2 * 512:(q2 + 1) * 512], psy[:])
        (nc.sync, nc.gpsimd)[oc % 2].dma_start(y_ext[oc, :, c], y_sb[:])

    # ======== qh=0 superblock (queries 0-1023) ========
    # s0: scores/exp h0 x low keys. The qkproj-hi matmuls and vproj-lo
    # fill the PE slack inside the stream. All DMA-gated blocks carry
    # tile_wait_until pins matching their REAL input-arrival times: the
    # tile scheduler's DMA model is optimistic, and without a pin it
    # front-loads these into the static engine order, where they then
    # stall the score->exp chain at runtime.
    for j in range(8):
        scores_chunk(0, j, 0)

    # qkproj-hi: k -> avA (freed first by softplus-hi's k exp, unblocking
    # the av(h0) accumulator), q -> avB halves. Pinned late enough that
    # the runtime xt-hi arrival (~26us) can't stall the score stream.
    psm[(1, 1)] = ps_pool.tile([128, QH], dt.float32, name="psm11", tag="avA")
    psm[(0, 1)] = [
        ps_pool.tile([128, 512], dt.float32, name=f"psm01_{q2}", tag=f"avB{q2}")
        for q2 in range(2)]
    for ki in range(KI):
        with tc.tile_wait_until(0.0145 + 0.002 * (ki // 2)):
            for p in (1, 0):
                for qc in range(2):
                    t = psm[(p, 1)]
                    if isinstance(t, list):
                        dst = t[qc][:, 0:512]
                    else:
                        dst = t[:, qc * 512:qc * 512 + 512]
                    nc.tensor.matmul(
                        dst,
                        wqk_sb[:, p, ki, :],
                        xt[:, 1, ki, qc * 512:(qc + 1) * 512],
                        start=(ki == 0), stop=(ki == KI - 1),
                    )

    # ACT: softplus-hi slots into the exp stream (k first: avA frees for
    # the av(h0) accumulator immediately).
    nc.scalar.activation(tmp[:, 1, QH:S], psm[(1, 1)][:], AF.Exp)
    for q2 in range(2):
        nc.scalar.activation(tmp[:, 0, QH + q2 * 512:QH + (q2 + 1) * 512],
                             psm[(0, 1)][q2][:], AF.Exp)
    nc.scalar.activation(mag[:, :, QH:S], tmp[:, :, QH:S], AF.Ln, bias=1.0)

    # DVE: hi-half trig + embeds (h0 first - its hi-key scores come at s2)
    with tc.tile_wait_until(0.028):
        tk_build(1)
    emb_mul(1, 0, qcols=full, kcols=full)
    emb_mul(1, 1, qcols=full, kcols=full)

    # s1: scores h1 x low keys only - the attn@v chunks are redistributed
    # into s2..qh1-s1, because v (whose wv input only lands ~27us) isn't
    # evicted until ~32us.
    for j in range(8):
        scores_chunk(1, j, 0)

    # vproj: wv lands ~27us; the avB banks free up once softplus-hi's q
    # exp has read the qkproj-hi psums.
    with tc.tile_wait_until(0.026):
        psv0 = v_group_mm(0, "avB0")
        psv1 = v_group_mm(1, "avB1")
    with tc.tile_wait_until(0.029):
        v_group_evict(0, psv0)
        v_group_evict(1, psv1)
    with tc.tile_wait_until(0.032):
        psv2 = v_group_mm(2, "avB0")
        psv3 = v_group_mm(3, "avB1")
    with tc.tile_wait_until(0.035):
        v_group_evict(2, psv2)
        v_group_evict(3, psv3)

    # s2: av(h0) x low keys + scores h0 x high keys
    av_ps[(0, 0)] = ps_pool.tile([128, QH], dt.float32, name="av0_0", tag="avA")
    for j in range(8):
        av_chunk(0, j, 0)
        scores_chunk(0, 8 + j, 0)

    # s3: av(h1) x low keys + scores h1 x high keys + av(h0) hi (lag 1)
    av_ps[(1, 0)] = [
        ps_pool.tile([128, 512], dt.float32, name=f"av1_0{q2}", tag=f"avB{q2}")
        for q2 in range(2)]
    for j in range(8):
        av_chunk(1, j, 0)
        scores_chunk(1, 8 + j, 0)
        if j >= 1:
            av_chunk(0, 8 + j - 1, 0)
    av_chunk(0, 15, 0)
    normalize(0, 0)

    # ======== qh=1 superblock (queries 1024-2047) ========
    # s0: scores h0 x low keys + av(h1) x qh0 high keys (lag 1); h1's
    # qh0 attn@v spills into qh1's PE slack instead of overcommitting s3
    for j in range(8):
        scores_chunk(0, j, 1)
        if j >= 1:
            av_chunk(1, 8 + j - 1, 0)
    av_chunk(1, 15, 0)
    normalize(1, 0)
    # s1: scores h1-lo + av(h0)-lo + the qh=0 output projection
    av_ps[(0, 1)] = ps_pool.tile([128, QH], dt.float32, name="av0_1", tag="avA")
    for j in range(8):
        scores_chunk(1, j, 1)
        av_chunk(0, j, 1)
        oproj(0, j)
    # s2: scores h0-hi + av(h1)-lo (both column halves)
    av_ps[(1, 1)] = [
        ps_pool.tile([128, 512], dt.float32, name=f"av1_1{q2}", tag=f"avB{q2}")
        for q2 in range(2)]
    for j in range(8):
        scores_chunk(0, 8 + j, 1)
        av_chunk(1, j, 1)
    # s3: scores h1-hi + av(h0)-hi + av(h1)-hi half 0 (lag 1), so half
    # 0's accumulator completes right after the last exp and its
    # normalize chain starts immediately; half 1's hi keys are deferred
    # to the tail as PE filler for the rowsum chain.
    for j in range(8):
        scores_chunk(1, 8 + j, 1)
        av_chunk(0, 8 + j, 1)
        if j >= 1:
            av_chunk(1, 8 + j - 1, 1, q2s=(0,))
    normalize(0, 1)
    av_chunk(1, 15, 1, q2s=(0,))

    # ---- tail. Both column halves' 1/rowsum chains pipeline across
    # DVE/DMA/ACT/GpSimd; the final output projection chases per half in
    # [128,512] pieces with evictions split DVE/ACT. ----
    rrf = persist.tile([1, QH], dt.bfloat16, name="rrf", tag="rrf")
    rsbf = persist.tile([D, QH], dt.bfloat16, name="rsbf", tag="rsbf")

    def norm_final_evict(hf):
        pavh = av_ps[(1, 1)][hf]
        cl = slice(hf * 512, (hf + 1) * 512)
        lnt = persist.tile([1, 512], dt.float32, name=f"lnt{hf}", tag=f"lnt{hf}")
        # ACT reads the PSUM rowsum row (partition 64) directly, writing to
        # partition 0 - skips the SBUF bounce + DMA relocation hop
        nc.scalar.activation(lnt[:], pavh[D:D + 1, :], AF.Ln)
        with nc.allow_low_precision(reason="softmax 1/rowsum in bf16, ~0.4%"):
            nc.scalar.activation(rrf[:, cl], lnt[:], AF.Exp, scale=-1.0)
        nc.gpsimd.partition_broadcast(rsbf[:, cl], rrf[:, cl])

    def norm_final_mul(hf):
        pavh = av_ps[(1, 1)][hf]
        cl = slice(hf * 512, (hf + 1) * 512)
        with nc.allow_low_precision(reason="pre-normalize attn@v in bf16, ~0.4%"):
            nc.vector.tensor_mul(
                outT[64:128, QH + hf * 512:QH + (hf + 1) * 512],
                pavh[0:D, :], rsbf[:, cl])

    def oproj_final_half(hf, tags):
        for oc in range(OC):
            psy = ps_pool.tile([128, 512], dt.float32, name=f"psyF{hf}_{oc}",
                               tag=tags[oc % len(tags)])
            nc.tensor.matmul(
                psy[:],
                wo_sb[:, oc * 128:(oc + 1) * 128],
                outT[:, QH + hf * 512:QH + (hf + 1) * 512],
                start=True, stop=True,
            )
            y_sb = out_pool.tile([128, 512], dt.bfloat16,
                                 name=f"yF{hf}_{oc}", tag="y")
            if oc % 2 == 0:
                nc.vector.tensor_copy(y_sb[:], psy[:])
            else:
                nc.scalar.activation(y_sb[:], psy[:], AF.Copy)
            (nc.sync, nc.gpsimd, nc.scalar)[oc % 3].dma_start(
                y_ext[oc, :, QH + hf * 512:QH + (hf + 1) * 512], y_sb[:])

    # PE fills the half-0 rowsum-chain window with half-1's attn@v; each
    # half's normalize + output projection starts as soon as ITS
    # accumulator (a separate PSUM tile) completes. Junk matmuls plug
    # the rowsum-chain waits so the PE clock-gate never drops off 2.4
    # GHz (an idle gap costs 2x on every tail matmul after it).
    def keep_warm(n, tag="scA"):
        wps = ps_pool.tile([128, 512], dt.float32, name=f"warmF{keep_warm.i}",
                           tag=tag)
        keep_warm.i += 1
        for i in range(n):
            nc.tensor.matmul(wps[:], warm_sb[:, 0:128], warm_sb[:],
                             start=(i == 0), stop=(i == n - 1))
    keep_warm.i = 0

    norm_final_evict(0)
    for kc in range(8, KC):
        av_chunk(1, kc, 1, q2s=(1,))
        keep_warm(1)
    norm_final_evict(1)
    # fill the rowsum-chain window (last av -> outT ready, ~3us) so the
    # PE clock-gate holds 2.4GHz into the final output projection
    keep_warm(12)
    norm_final_mul(0)
    oproj_final_half(0, ("scB", "avA", "scA"))
    norm_final_mul(1)
    oproj_final_half(1, ("scB", "avA", "scA", "avB0"))


def _build():
    with _patch_act_tables():
        return _build_inner()


def _build_inner():
    nc = bacc.Bacc()
    dt = mybir.dt

    ext = (
        nc.declare_dram_parameter("xt", [128, 2, KI, QH], dt.bfloat16, isOutput=False),
        nc.declare_dram_parameter("wqk", [128, 2, KI, ED], dt.bfloat16, isOutput=False),
        nc.declare_dram_parameter("wv", [128, KI, ED], dt.bfloat16, isOutput=False),
        nc.declare_dram_parameter("tq", [128, 2, 2, QH], dt.bfloat16, isOutput=False),
        nc.declare_dram_parameter("btr", [128, 2], dt.float32, isOutput=False),
        nc.declare_dram_parameter("woT", [128, DIM], dt.bfloat16, isOutput=False),
        nc.declare_dram_parameter("yT", [OC, 128, S], dt.bfloat16, isOutput=True),
    )

    with tile.TileContext(nc) as tc:
        with tc.tile_pool(name="persist", bufs=1) as persist, \
             tc.tile_pool(name="ps", bufs=1, space="PSUM") as ps_pool, \
             tc.tile_pool(name="out", bufs=6) as out_pool, \
             tc.tile_pool(name="xtp", bufs=1) as xt_pool, \
             tc.tile_pool(name="expp", bufs=2) as exp_pool:
            _build_body(nc, tc, persist, ps_pool, out_pool, xt_pool, exp_pool, ext)
    nc.compile()
    return nc


def _get_nc():
    global _compiled_nc
    if _compiled_nc is None:
        _compiled_nc = _build()
    return _compiled_nc


def _prep_inputs(x, wq, wk, wv, wo, pope_bias):
    """Host-side sharding + layout prep. Returns in_maps for the 8 cores."""
    x2 = np.ascontiguousarray(x.reshape(S, DIM).astype(np.float32))

    # trig tables (f64 phases for accuracy); heads share cos/sin(freqs)
    inv = 10000.0 ** (-(np.arange(D, dtype=np.float64) / D))
    pos = np.arange(S, dtype=np.float64)
    freqs = pos[:, None] * inv[None, :]                       # [S, D]
    bias = np.clip(pope_bias.astype(np.float64), -2 * math.pi, 0.0)  # [H, D]

    # tq[p, half, t, c]: cos/sin(freqs[half*QH + c, p%64])
    tq = np.empty((128, 2, 2, QH), BF16)
    ct = np.cos(freqs).T.reshape(D, 2, QH)                    # [64, half, QH]
    st = np.sin(freqs).T.reshape(D, 2, QH)
    tq[0:64, :, 0] = ct
    tq[64:128, :, 0] = ct
    tq[0:64, :, 1] = st
    tq[64:128, :, 1] = st

    # xt[q, half, ki, c] = x[half*QH + c, ki*128 + q]; 16KB contiguous
    # per (q, half) so the DMA moves whole halves as single big-run pieces
    xt = np.ascontiguousarray(
        x2.T.reshape(KI, 128, 2, QH).transpose(1, 2, 0, 3)).astype(BF16)

    in_maps = []
    for c in range(NCORES):
        hs = slice(c * HPC * D, (c + 1) * HPC * D)            # head-channel slice
        wqk = np.empty((128, 2, KI, ED), BF16)
        for p, wm in enumerate((wq, wk)):
            wt = np.ascontiguousarray(wm[hs, :].astype(np.float32).T)  # [DIM, ED]
            wqk[:, p] = wt.reshape(KI, 128, ED).transpose(1, 0, 2)
        wvt = np.ascontiguousarray(wv[hs, :].astype(np.float32).T)
        wv_m = np.ascontiguousarray(
            wvt.reshape(KI, 128, ED).transpose(1, 0, 2)).astype(BF16)

        # per-head bias trig for on-chip angle addition, head-major rows:
        # partition 64h+d holds (cos(bias[h,d]), sin(bias[h,d]))
        btr = np.empty((128, 2), np.float32)
        for h in range(HPC):
            b = bias[c * HPC + h]                              # [D]
            btr[64 * h:64 * h + 64, 0] = np.cos(b)
            btr[64 * h:64 * h + 64, 1] = np.sin(b)

        woT = np.ascontiguousarray(wo[:, hs].astype(np.float32).T).astype(BF16)

        in_maps.append({
            "xt": xt, "wqk": wqk, "wv": wv_m, "tq": tq, "btr": btr,
            "woT": woT,
        })
    return in_maps


def kernel(x, wq, wk, wv, wo, pope_bias):
    nc = _get_nc()
    in_maps = _prep_inputs(np.asarray(x), np.asarray(wq), np.asarray(wk),
                           np.asarray(wv), np.asarray(wo), np.asarray(pope_bias))
    res = run_bass_kernel_spmd(nc, in_maps, list(range(NCORES)))
    y = np.zeros((DIM, S), np.float32)
    for c in range(NCORES):
        y += res.results[c]["yT"].reshape(DIM, S).astype(np.float32)
    return np.ascontiguousarray(y.T).reshape(1, S, DIM)


if __name__ == "__main__":
    rng = np.random.default_rng(0)
    out = kernel(
        x=rng.standard_normal((1, S, DIM)).astype(np.float32),
        wq=rng.standard_normal((DIM, DIM)).astype(np.float32) / 32,
        wk=rng.standard_normal((DIM, DIM)).astype(np.float32) / 32,
        wv=rng.standard_normal((DIM, DIM)).astype(np.float32) / 32,
        wo=rng.standard_normal((DIM, DIM)).astype(np.float32) / 32,
        pope_bias=-rng.random((H, D), np.float32) * 3.0,
    )
    print("out", out.shape, out.dtype, np.abs(out).mean())